# revision 1
# baseline (speedup 1.0000x reference)
"""Causal multi-head attention (B=2, S=2048, H=1024, 16 heads, hd=64) on 8
Trainium2 NeuronCores.

Sharding: batch x head-group. Core c handles batch c//4 and the 4 heads
4*(c%4)..4*(c%4)+3 (a 256-wide column slice of Q/K/V). Each core computes its
heads' contribution to the output projection (row-parallel Wo); the host sums
the 4 partials per batch and adds bo.

Per-core kernel (all matmuls in float32r = fp32 storage, TF32-like PE mode):
  phase 1: qT/kT = W.T-slice.T @ xT (+bias via K=1 matmul), v natural
           (lhsT = xT). xT = hidden[b].T is prepared host-side, so no
           on-device transposes anywhere.
  phase 2: per (head, 512-query block): scoresT[k,q] tiles on PE (causal:
           only k-blocks <= block end), -1e9 staircase mask added to PSUM on
           DVE for diagonal tiles, exp on ACT (scale=1/8 folded in; scores
           are bounded ~|3.8| so no max-subtraction is needed), then
           ctxT[65,q] = v_aug.T @ expT accumulated on PE - v_aug carries a
           ones column so row 64 is the softmax denominator. Reciprocal row
           is broadcast across 64 partitions with a K=1 matmul and applied
           on DVE, writing normalized ctxT straight into the outproj
           stationary layout.
  phase 3: out_partial[tok,1024] = ctxT.T @ WoT-slice, streamed to DRAM.
"""
import numpy as np

import concourse.bass as bass
import concourse.mybir as mybir
import concourse.tile as tile
from concourse.bass import ts
from concourse.bass_utils import run_bass_kernel_spmd

B, S, H, NH, HD = 2, 2048, 1024, 16, 64
NCORES = 8
HPC = 4            # heads per core
HSW = HPC * HD     # 256: head-slice width
F32 = mybir.dt.float32
F32R = mybir.dt.float32r
NEG = -1.0e9


def _split_multi_waits(nc) -> int:
    """This walrus accepts at most ONE sync wait per instruction. Split any
    multi-wait instruction into single-wait NOPs (same engine, just before
    it) + the instruction carrying the last wait. Equivalent semantics:
    waits run in program order on the engine's queue."""
    n = 0
    for f in nc.m.functions:
        for blk in f.blocks:
            new_insts = []
            for inst in blk.instructions:
                si = inst.sync_info
                if si is not None and si.on_wait and len(si.on_wait) > 1:
                    waits = list(si.on_wait)
                    for i, w in enumerate(waits[:-1]):
                        new_insts.append(mybir.InstNoOp(
                            name=f"{inst.name}-ws{i}",
                            engine=inst.engine,
                            bass_nofuse=True,
                            sync_info=mybir.SyncInfo(on_wait=[w], on_update=[]),
                        ))
                        n += 1
                    si.on_wait = [waits[-1]]
                new_insts.append(inst)
            blk.instructions[:] = new_insts
    return n


def _build():
    nc = bass.Bass()
    xt_d = nc.dram_tensor("xt", [H, S], F32R, kind="ExternalInput")
    wq_d = nc.dram_tensor("wq", [H, HSW], F32R, kind="ExternalInput")
    wk_d = nc.dram_tensor("wk", [H, HSW], F32R, kind="ExternalInput")
    wv_d = nc.dram_tensor("wv", [H, HSW], F32R, kind="ExternalInput")
    wo_d = nc.dram_tensor("wo", [HSW, H], F32R, kind="ExternalInput")
    bqkv_d = nc.dram_tensor("bqkv", [3, HSW], F32R, kind="ExternalInput")
    mb_d = nc.dram_tensor("mb", [128, 4, 512], F32, kind="ExternalInput")
    out_d = nc.dram_tensor("out", [S, H], F32, kind="ExternalOutput")

    EXP = mybir.ActivationFunctionType.Exp
    NQB = S // 512      # 4 query blocks per head
    NTC = S // 128      # 16 token chunks

    with tile.TileContext(nc) as tc:
        with tc.tile_pool(name="const", bufs=1) as constp, \
             tc.tile_pool(name="persist", bufs=1) as pers:
            wq = constp.tile([128, 8, HSW], F32R)
            wk = constp.tile([128, 8, HSW], F32R)
            wv = constp.tile([128, 8, HSW], F32R)
            wo = constp.tile([128, 2, H], F32R)
            bqkv = constp.tile([1, 3, HSW], F32R)
            mb = constp.tile([128, 4, 512], F32)
            onesf = constp.tile([128, 512], F32)
            nc.vector.memset(onesf, 1.0)
            ones = constp.tile([1, 512], F32R)
            nc.vector.tensor_copy(out=ones, in_=onesf[0:1, :])

            qT = pers.tile([128, 2, S], F32R)     # [2 heads x 64 hd, mchunk, tok]
            kT = pers.tile([128, 2, S], F32R)
            vaug = pers.tile([128, 4, NTC, HD + 1], F32R)  # [ktok, head, kchunk, hd|1]
            ctxT = pers.tile([128, 2, S], F32R)   # outproj stationary layout
            nc.vector.tensor_copy(
                out=vaug[:, :, :, HD:HD + 1],
                in_=onesf[:, 0:64].rearrange("p (a b o) -> p a b o", a=4, b=16))

            # ---- phase 1: projections ----
            with tc.tile_pool(name="xtp", bufs=1) as xtp, \
                 tc.tile_pool(name="ps1", bufs=3, space="PSUM") as ps1, \
                 tc.tile_pool(name="ps1v", bufs=3, space="PSUM") as ps1v:
                # DMA order: first xt chunks + wq unblock the first matmuls;
                # bulk weights follow.
                # xt at (kc, nb) granularity: the first qk accumulation
                # chain (nb=0) can start after 256KB instead of 1MB/chunk.
                xt = xtp.tile([128, 8, S], F32R)
                for kc in range(8):
                    nc.sync.dma_start(out=xt[:, kc, ts(0, 512)],
                                      in_=xt_d[ts(kc, 128), ts(0, 512)])
                nc.sync.dma_start(out=wq, in_=wq_d[:, :].rearrange("(c p) n -> p c n", p=128))
                nc.sync.dma_start(out=wk, in_=wk_d[:, :].rearrange("(c p) n -> p c n", p=128))
                nc.sync.dma_start(out=bqkv, in_=bqkv_d[:, :].rearrange("(o r) n -> o r n", o=1))
                for nb in range(1, NQB):
                    for kc in range(8):
                        nc.sync.dma_start(out=xt[:, kc, ts(nb, 512)],
                                          in_=xt_d[ts(kc, 128), ts(nb, 512)])
                nc.sync.dma_start(out=wv, in_=wv_d[:, :].rearrange("(c p) n -> p c n", p=128))
                nc.sync.dma_start(out=mb, in_=mb_d[:, :, :])
                nc.sync.dma_start(out=wo, in_=wo_d[:, :].rearrange("(c p) n -> p c n", p=128))

                for w, brow, dst in ((wq, 0, qT), (wk, 1, kT)):
                    for nb in range(NQB):
                        ps = ps1.tile([128, 512], F32, tag="ps1")
                        for kc in range(8):
                            nc.tensor.matmul(ps, w[:, kc, 0:128],
                                             xt[:, kc, ts(nb, 512)],
                                             start=(kc == 0), stop=False)
                        nc.tensor.matmul(ps, bqkv[0:1, brow, 0:128],
                                         ones[0:1, 0:512], start=False, stop=True)
                        nc.vector.tensor_copy(out=dst[:, 0, ts(nb, 512)], in_=ps)

                for t in range(NTC):
                    ps = ps1v.tile([128, HSW], F32, tag="psv")
                    for kc in range(8):
                        nc.tensor.matmul(ps, xt[:, kc, ts(t, 128)], wv[:, kc, :],
                                         start=(kc == 0), stop=False)
                    nc.tensor.matmul(ps, ones[0:1, 0:128], bqkv[0:1, 2, :],
                                     start=False, stop=True)
                    for h in range(HPC):
                        nc.vector.tensor_copy(out=vaug[:, h, t, 0:HD],
                                              in_=ps[:, ts(h, HD)])

            # ---- phase 2+3: attention with software-pipelined epilogues ----
            # Heads processed singly (qb outer). The normalization epilogue of
            # slot (qb, h) -- reciprocal via ACT exp(-ln d), PE broadcast, DVE
            # scale -- is DEFERRED until after the NEXT slot's score/ctx groups
            # are emitted: the static per-engine schedule then has the next
            # slot's matmuls between ctx-stop and the broadcast matmul, so the
            # PE never idles waiting on the reciprocal chain. Outproj for qb is
            # deferred two slots for the same reason.
            # PSUM: pss 2x2 + ctx 3 + misc 1 = 8 banks.
            with tc.tile_pool(name="pss", bufs=2, space="PSUM") as pss, \
                 tc.tile_pool(name="psc", bufs=2, space="PSUM") as psc, \
                 tc.tile_pool(name="psm", bufs=1, space="PSUM") as psm, \
                 tc.tile_pool(name="ps1b", bufs=1, space="PSUM") as ps1b, \
                 tc.tile_pool(name="xtbp", bufs=2) as xtbp, \
                 tc.tile_pool(name="attnp", bufs=3) as attnp, \
                 tc.tile_pool(name="outp", bufs=3) as outp:

                # Interleaved heads-2/3 q/k projection chunks: dense 9-matmul
                # accumulation runs that keep the PE HAM activity window busy
                # (warm clock) between attention slots. xt was freed with
                # phase 1, so each chunk re-DMAs its [128c x 512] slice.
                def qk_mc1_chunk(w, brow, dst, nb):
                    def run():
                        xtb = xtbp.tile([128, 8, 512], F32R, tag="xtb",
                                        name="xtb")
                        for kc in range(8):
                            nc.sync.dma_start(
                                out=xtb[:, kc, :],
                                in_=xt_d[ts(kc, 128), ts(nb, 512)])
                        ps = ps1b.tile([128, 512], F32, tag="ps1b", name="ps1b")
                        for kc in range(8):
                            nc.tensor.matmul(ps, w[:, kc, 128:256],
                                             xtb[:, kc, :],
                                             start=(kc == 0), stop=False)
                        nc.tensor.matmul(ps, bqkv[0:1, brow, 128:256],
                                         ones[0:1, 0:512], start=False, stop=True)
                        nc.vector.tensor_copy(out=dst[:, 1, ts(nb, 512)], in_=ps)
                    return run
                qk_units = [qk_mc1_chunk(w, brow, dst, nb)
                            for (w, brow, dst) in ((wq, 0, qT), (wk, 1, kT))
                            for nb in range(NQB)]

                def norm_epilogue(qb, h, cps):
                    def run():
                        mc, ro = h // 2, (h % 2) * HD
                        lnr = attnp.tile([1, 512], F32, tag="lnr", name="lnr")
                        nc.scalar.activation(out=lnr, in_=cps[HD:HD + 1, :],
                                             func=mybir.ActivationFunctionType.Ln)
                        rec = attnp.tile([1, 512], F32R, tag="rec", name="rec")
                        nc.scalar.activation(out=rec, in_=lnr, func=EXP,
                                             scale=-1.0)
                        bps = psm.tile([128, 512], F32, tag="m", name="bps")
                        nc.tensor.matmul(bps[0:HD, :], ones[0:1, 0:HD],
                                         rec[0:1, :], start=True, stop=True)
                        bsb = attnp.tile([HD, 512], F32R, tag="bsb", name="bsb")
                        nc.vector.tensor_copy(out=bsb, in_=bps[0:HD, :])
                        nc.vector.tensor_mul(
                            out=ctxT[ro:ro + HD, mc, ts(qb, 512)],
                            in0=cps[0:HD, :], in1=bsb)
                    return run

                def outproj(qb):
                    def run():
                        for t in range(4 * qb, 4 * qb + 4):
                            osb = outp.tile([128, H], F32, tag="osb", name="osb")
                            for n2 in range(2):
                                # Alternate across two PSUM banks (ps1b is idle
                                # once the qk bursts are done) so each outproj
                                # blob is a dense 8-matmul run - it both flows
                                # faster and re-warms the HAM clock.
                                ops = (psm if n2 == 0 else ps1b).tile(
                                    [128, 512], F32,
                                    tag="m" if n2 == 0 else "ps1b", name="ops")
                                nc.tensor.matmul(ops, ctxT[:, 0, ts(t, 128)],
                                                 wo[:, 0, ts(n2, 512)],
                                                 start=True, stop=False)
                                nc.tensor.matmul(ops, ctxT[:, 1, ts(t, 128)],
                                                 wo[:, 1, ts(n2, 512)],
                                                 start=False, stop=True)
                                nc.vector.tensor_copy(out=osb[:, ts(n2, 512)],
                                                      in_=ops)
                            nc.sync.dma_start(out=out_d[ts(t, 128), :], in_=osb)
                    return run

                deferred = []
                for qb, h in ([(q, hh) for q in range(NQB) for hh in (0, 1)]
                              + [(q, hh) for q in range(NQB) for hh in (2, 3)]):
                    last_kb = 4 * qb + 3
                    if True:
                        mc, ro = h // 2, (h % 2) * HD
                        cps = psc.tile([128, 512], F32, tag="ctx", name="cps")
                        # Emit group PAIRS: 4 scores mms, 2 exps, 4 ctx mms.
                        # The PE stream becomes continuous ~8-matmul dense runs
                        # (scores of pair N overlap exp of pair N-1), which
                        # keeps the HAM activity window busy (full clock).
                        for gp in range(qb + 1):
                            sets = []
                            for g in (2 * gp, 2 * gp + 1):
                                sps = pss.tile([128, 2, 512], F32, tag="s",
                                               name="sps")
                                et = attnp.tile([128, 2, 512], F32R, tag="et",
                                                name="et")
                                sets.append((g, sps, et))
                                for u in range(2):
                                    kb = 2 * g + u
                                    nc.tensor.matmul(
                                        sps[:, u, :],
                                        kT[ro:ro + HD, mc, ts(kb, 128)],
                                        qT[ro:ro + HD, mc, ts(qb, 512)],
                                        start=True, stop=True)
                                    j = kb - 4 * qb
                                    if j >= 0:
                                        nc.vector.tensor_add(sps[:, u, :],
                                                             sps[:, u, :],
                                                             mb[:, j, :])
                            for g, sps, et in sets:
                                nc.scalar.activation(out=et, in_=sps, func=EXP,
                                                     scale=0.125)
                            for g, sps, et in sets:
                                for u in range(2):
                                    kb = 2 * g + u
                                    nc.tensor.matmul(cps[0:HD + 1, :],
                                                     vaug[:, h, kb, :],
                                                     et[:, u, :],
                                                     start=(kb == 0),
                                                     stop=(kb == last_kb))
                        # flush one pending epilogue, then defer this slot's
                        while len(deferred) >= 2:
                            deferred.pop(0)()
                        deferred.append(norm_epilogue(qb, h, cps))
                        # One dense 9-matmul projection burst after each
                        # heads-0/1 slot keeps the PE HAM window busy (warm
                        # clock); their heads-2/3 consumers are a half-section
                        # away so the bursts never serialize the pipeline.
                        if h <= 1 and qk_units:
                            qk_units.pop(0)()
                        if h == 3 and qb > 0:
                            deferred.append(outproj(qb - 1))
                for fn in deferred:
                    fn()
                outproj(NQB - 1)()

    _split_multi_waits(nc)
    return nc


_NC_CACHE = []


def _get_nc():
    if not _NC_CACHE:
        _NC_CACHE.append(_build())
    return _NC_CACHE[0]


def _staircase_mask() -> np.ndarray:
    """mb[p, j, f] = 0 where k<=q for diagonal tile j, else NEG.
    Allowed iff p <= f - 128*j (q = qb*512+f, k = qb*512+128*j+p)."""
    p = np.arange(128)[:, None, None]
    j = np.arange(4)[None, :, None]
    f = np.arange(512)[None, None, :]
    return np.where(p <= f - 128 * j, 0.0, NEG).astype(np.float32)


def _in_maps(inputs: dict) -> list[dict]:
    x = np.ascontiguousarray(np.asarray(inputs["hidden_states"], dtype=np.float32))
    Wq = np.asarray(inputs["Wq"], dtype=np.float32)
    Wk = np.asarray(inputs["Wk"], dtype=np.float32)
    Wv = np.asarray(inputs["Wv"], dtype=np.float32)
    Wo = np.asarray(inputs["Wo"], dtype=np.float32)
    bq = np.asarray(inputs["bq"], dtype=np.float32)
    bk = np.asarray(inputs["bk"], dtype=np.float32)
    bv = np.asarray(inputs["bv"], dtype=np.float32)

    xts = [np.ascontiguousarray(x[b].T) for b in range(B)]
    mb = _staircase_mask()
    maps = []
    for c in range(NCORES):
        b, hg = c // 4, c % 4
        hs = slice(hg * HSW, (hg + 1) * HSW)
        maps.append({
            "xt": xts[b],
            "wq": np.ascontiguousarray(Wq[hs, :].T),
            "wk": np.ascontiguousarray(Wk[hs, :].T),
            "wv": np.ascontiguousarray(Wv[hs, :].T),
            "wo": np.ascontiguousarray(Wo[:, hs].T),
            "bqkv": np.ascontiguousarray(np.stack([bq[hs], bk[hs], bv[hs]])),
            "mb": mb,
        })
    return maps


def run(inputs: dict, **spmd_kwargs):
    """Returns (full_output, BassKernelResults)."""
    nc = _get_nc()
    res = run_bass_kernel_spmd(nc, _in_maps(inputs), list(range(NCORES)),
                               **spmd_kwargs)
    bo = np.asarray(inputs["bo"], dtype=np.float32)
    out = np.empty((B, S, H), dtype=np.float32)
    for b in range(B):
        acc = res.results[4 * b]["out"].astype(np.float32)
        for hg in range(1, 4):
            acc = acc + res.results[4 * b + hg]["out"]
        out[b] = acc + bo
    return out, res


def kernel(**inputs) -> np.ndarray:
    out, _ = run(inputs)
    return out



# revision 10
# speedup vs baseline: 1.3849x; 1.3849x over previous
"""Causal multi-head attention (B=2, S=2048, H=1024, 16 heads, hd=64) on 8
Trainium2 NeuronCores.

Sharding: batch x head-group. Core c handles batch c//4 and the 4 heads
4*(c%4)..4*(c%4)+3 (a 256-wide column slice of Q/K/V). Each core computes its
heads' contribution to the output projection (row-parallel Wo); the host sums
the 4 partials per batch and adds bo.

Per-core kernel (all matmuls in bf16 (fp32 PSUM accumulation)):
  phase 1: qT/kT = W.T-slice.T @ xT (+bias via K=1 matmul), v natural
           (lhsT = xT). xT = hidden[b].T is prepared host-side, so no
           on-device transposes anywhere.
  phase 2: per (head, 512-query block): scoresT[k,q] tiles on PE (causal:
           only k-blocks <= block end), -1e9 staircase mask added to PSUM on
           DVE for diagonal tiles, exp on ACT (scale=1/8 folded in; scores
           are bounded ~|3.8| so no max-subtraction is needed), then
           ctxT[65,q] = v_aug.T @ expT accumulated on PE - v_aug carries a
           ones column so row 64 is the softmax denominator. Reciprocal row
           is broadcast across 64 partitions with a K=1 matmul and applied
           on DVE, writing normalized ctxT straight into the outproj
           stationary layout.
  phase 3: out_partial[tok,1024] = ctxT.T @ WoT-slice, streamed to DRAM.
"""
import ml_dtypes
import numpy as np

import concourse.bass as bass
import concourse.mybir as mybir
import concourse.tile as tile
from concourse.bass import ts
from concourse.bass_utils import run_bass_kernel_spmd

B, S, H, NH, HD = 2, 2048, 1024, 16, 64
NCORES = 8
HPC = 4            # heads per core
HSW = HPC * HD     # 256: head-slice width
F32 = mybir.dt.float32
BF16 = mybir.dt.bfloat16
NEG = -1.0e9


def _split_multi_waits(nc) -> int:
    """This walrus accepts at most ONE sync wait per instruction. Split any
    multi-wait instruction into single-wait NOPs (same engine, just before
    it) + the instruction carrying the last wait. Equivalent semantics:
    waits run in program order on the engine's queue."""
    n = 0
    for f in nc.m.functions:
        for blk in f.blocks:
            new_insts = []
            for inst in blk.instructions:
                si = inst.sync_info
                if si is not None and si.on_wait and len(si.on_wait) > 1:
                    waits = list(si.on_wait)
                    for i, w in enumerate(waits[:-1]):
                        new_insts.append(mybir.InstNoOp(
                            name=f"{inst.name}-ws{i}",
                            engine=inst.engine,
                            bass_nofuse=True,
                            sync_info=mybir.SyncInfo(on_wait=[w], on_update=[]),
                        ))
                        n += 1
                    si.on_wait = [waits[-1]]
                new_insts.append(inst)
            blk.instructions[:] = new_insts
    return n


def _build():
    nc = bass.Bass()
    xt_d = nc.dram_tensor("xt", [H, S], BF16, kind="ExternalInput")
    wq_d = nc.dram_tensor("wq", [H, HSW], BF16, kind="ExternalInput")
    wk_d = nc.dram_tensor("wk", [H, HSW], BF16, kind="ExternalInput")
    wv_d = nc.dram_tensor("wv", [H, HSW], BF16, kind="ExternalInput")
    wo_d = nc.dram_tensor("wo", [HSW, H], BF16, kind="ExternalInput")
    bqkv_d = nc.dram_tensor("bqkv", [3, HSW], BF16, kind="ExternalInput")
    mb_d = nc.dram_tensor("mb", [128, 4, 512], F32, kind="ExternalInput")
    out_d = nc.dram_tensor("out", [S, H], F32, kind="ExternalOutput")

    EXP = mybir.ActivationFunctionType.Exp
    NQB = S // 512      # 4 query blocks per head
    NTC = S // 128      # 16 token chunks

    with tile.TileContext(nc) as tc:
        with tc.tile_pool(name="const", bufs=1) as constp, \
             tc.tile_pool(name="persist", bufs=1) as pers:
            wq = constp.tile([128, 8, HSW], BF16)
            wk = constp.tile([128, 8, HSW], BF16)
            wv = constp.tile([128, 8, HSW], BF16)
            wo = constp.tile([128, 2, H], BF16)
            bqkv = constp.tile([1, 3, HSW], BF16)
            mb = constp.tile([128, 4, 512], F32)
            onesf = constp.tile([128, 512], F32)
            nc.vector.memset(onesf, 1.0)
            ones = constp.tile([1, 512], BF16)
            nc.vector.tensor_copy(out=ones, in_=onesf[0:1, :])

            qT = pers.tile([128, 2, S], BF16)     # [2 heads x 64 hd, mchunk, tok]
            kT = pers.tile([128, 2, S], BF16)
            vaug = pers.tile([128, 4, NTC, HD + 1], BF16)  # [ktok, head, kchunk, hd|1]
            ctxT = pers.tile([128, 2, S], BF16)   # outproj stationary layout
            nc.vector.tensor_copy(
                out=vaug[:, :, :, HD:HD + 1],
                in_=onesf[:, 0:64].rearrange("p (a b o) -> p a b o", a=4, b=16))

            # ---- phase 1: projections ----
            with tc.tile_pool(name="xtp", bufs=1) as xtp, \
                 tc.tile_pool(name="ps1", bufs=3, space="PSUM") as ps1, \
                 tc.tile_pool(name="ps1v", bufs=3, space="PSUM") as ps1v:
                # DMA order: first xt chunks + wq unblock the first matmuls;
                # bulk weights follow.
                # xt at (kc, nb) granularity: the first qk accumulation
                # chain (nb=0) can start after 256KB instead of 1MB/chunk.
                xt = xtp.tile([128, 8, S], BF16)
                for kc in range(8):
                    nc.sync.dma_start(out=xt[:, kc, ts(0, 512)],
                                      in_=xt_d[ts(kc, 128), ts(0, 512)])
                nc.sync.dma_start(out=wq, in_=wq_d[:, :].rearrange("(c p) n -> p c n", p=128))
                nc.sync.dma_start(out=wk, in_=wk_d[:, :].rearrange("(c p) n -> p c n", p=128))
                nc.sync.dma_start(out=bqkv, in_=bqkv_d[:, :].rearrange("(o r) n -> o r n", o=1))
                for nb in range(1, NQB):
                    for kc in range(8):
                        nc.sync.dma_start(out=xt[:, kc, ts(nb, 512)],
                                          in_=xt_d[ts(kc, 128), ts(nb, 512)])
                nc.sync.dma_start(out=wv, in_=wv_d[:, :].rearrange("(c p) n -> p c n", p=128))
                nc.sync.dma_start(out=mb, in_=mb_d[:, :, :])
                nc.sync.dma_start(out=wo, in_=wo_d[:, :].rearrange("(c p) n -> p c n", p=128))

                for w, brow, dst in ((wq, 0, qT), (wk, 1, kT)):
                    for nb in range(NQB):
                        ps = ps1.tile([128, 512], F32, tag="ps1")
                        for kc in range(8):
                            nc.tensor.matmul(ps, w[:, kc, 0:128],
                                             xt[:, kc, ts(nb, 512)],
                                             start=(kc == 0), stop=False)
                        nc.tensor.matmul(ps, bqkv[0:1, brow, 0:128],
                                         ones[0:1, 0:512], start=False, stop=True)
                        nc.vector.tensor_copy(out=dst[:, 0, ts(nb, 512)], in_=ps)

                for t in range(NTC):
                    ps = ps1v.tile([128, HSW], F32, tag="psv")
                    for kc in range(8):
                        nc.tensor.matmul(ps, xt[:, kc, ts(t, 128)], wv[:, kc, :],
                                         start=(kc == 0), stop=False)
                    nc.tensor.matmul(ps, ones[0:1, 0:128], bqkv[0:1, 2, :],
                                     start=False, stop=True)
                    for h in range(HPC):
                        nc.vector.tensor_copy(out=vaug[:, h, t, 0:HD],
                                              in_=ps[:, ts(h, HD)])

            # ---- phase 2+3: attention with software-pipelined epilogues ----
            # Heads processed singly (qb outer). The normalization epilogue of
            # slot (qb, h) -- reciprocal via ACT exp(-ln d), PE broadcast, DVE
            # scale -- is DEFERRED until after the NEXT slot's score/ctx groups
            # are emitted: the static per-engine schedule then has the next
            # slot's matmuls between ctx-stop and the broadcast matmul, so the
            # PE never idles waiting on the reciprocal chain. Outproj for qb is
            # deferred two slots for the same reason.
            # PSUM: pss 2x2 + ctx 3 + misc 1 = 8 banks.
            with tc.tile_pool(name="pss", bufs=2, space="PSUM") as pss, \
                 tc.tile_pool(name="psc", bufs=2, space="PSUM") as psc, \
                 tc.tile_pool(name="psm", bufs=1, space="PSUM") as psm, \
                 tc.tile_pool(name="ps1b", bufs=1, space="PSUM") as ps1b, \
                 tc.tile_pool(name="xtbp", bufs=2) as xtbp, \
                 tc.tile_pool(name="attnp", bufs=3) as attnp, \
                 tc.tile_pool(name="outp", bufs=3) as outp:

                # Interleaved heads-2/3 q/k projection chunks: dense 9-matmul
                # accumulation runs that keep the PE HAM activity window busy
                # (warm clock) between attention slots. xt was freed with
                # phase 1, so each chunk re-DMAs its [128c x 512] slice.
                def qk_mc1_chunk(w, brow, dst, nb):
                    def run():
                        xtb = xtbp.tile([128, 8, 512], BF16, tag="xtb",
                                        name="xtb")
                        for kc in range(8):
                            nc.sync.dma_start(
                                out=xtb[:, kc, :],
                                in_=xt_d[ts(kc, 128), ts(nb, 512)])
                        ps = ps1b.tile([128, 512], F32, tag="ps1b", name="ps1b")
                        for kc in range(8):
                            nc.tensor.matmul(ps, w[:, kc, 128:256],
                                             xtb[:, kc, :],
                                             start=(kc == 0), stop=False)
                        nc.tensor.matmul(ps, bqkv[0:1, brow, 128:256],
                                         ones[0:1, 0:512], start=False, stop=True)
                        nc.vector.tensor_copy(out=dst[:, 1, ts(nb, 512)], in_=ps)
                    return run
                qk_units = [qk_mc1_chunk(w, brow, dst, nb)
                            for (w, brow, dst) in ((wq, 0, qT), (wk, 1, kT))
                            for nb in range(NQB)]

                def norm_epilogue(qb, h, cps):
                    def run():
                        mc, ro = h // 2, (h % 2) * HD
                        lnr = attnp.tile([1, 512], F32, tag="lnr", name="lnr")
                        nc.scalar.activation(out=lnr, in_=cps[HD:HD + 1, :],
                                             func=mybir.ActivationFunctionType.Ln)
                        rec = attnp.tile([1, 512], BF16, tag="rec", name="rec")
                        nc.scalar.activation(out=rec, in_=lnr, func=EXP,
                                             scale=-1.0)
                        bps = psm.tile([128, 512], F32, tag="m", name="bps")
                        nc.tensor.matmul(bps[0:HD, :], ones[0:1, 0:HD],
                                         rec[0:1, :], start=True, stop=True)
                        bsb = attnp.tile([HD, 512], BF16, tag="bsb", name="bsb")
                        nc.vector.tensor_copy(out=bsb, in_=bps[0:HD, :])
                        nc.vector.tensor_mul(
                            out=ctxT[ro:ro + HD, mc, ts(qb, 512)],
                            in0=cps[0:HD, :], in1=bsb)
                    return run

                def outproj(qb):
                    def run():
                        for t in range(4 * qb, 4 * qb + 4):
                            osb = outp.tile([128, H], F32, tag="osb", name="osb")
                            for n2 in range(2):
                                # Alternate across two PSUM banks (ps1b is idle
                                # once the qk bursts are done) so each outproj
                                # blob is a dense 8-matmul run - it both flows
                                # faster and re-warms the HAM clock.
                                ops = (psm if n2 == 0 else ps1b).tile(
                                    [128, 512], F32,
                                    tag="m" if n2 == 0 else "ps1b", name="ops")
                                nc.tensor.matmul(ops, ctxT[:, 0, ts(t, 128)],
                                                 wo[:, 0, ts(n2, 512)],
                                                 start=True, stop=False)
                                nc.tensor.matmul(ops, ctxT[:, 1, ts(t, 128)],
                                                 wo[:, 1, ts(n2, 512)],
                                                 start=False, stop=True)
                                nc.vector.tensor_copy(out=osb[:, ts(n2, 512)],
                                                      in_=ops)
                            nc.sync.dma_start(out=out_d[ts(t, 128), :], in_=osb)
                    return run

                deferred = []
                for qb, h in ([(q, hh) for q in range(NQB) for hh in (0, 1)]
                              + [(q, hh) for q in range(NQB) for hh in (2, 3)]):
                    last_kb = 4 * qb + 3
                    if True:
                        mc, ro = h // 2, (h % 2) * HD
                        cps = psc.tile([128, 512], F32, tag="ctx", name="cps")
                        # Emit group PAIRS: 4 scores mms, 2 exps, 4 ctx mms.
                        # The PE stream becomes continuous ~8-matmul dense runs
                        # (scores of pair N overlap exp of pair N-1), which
                        # keeps the HAM activity window busy (full clock).
                        for gp in range(qb + 1):
                            sets = []
                            for g in (2 * gp, 2 * gp + 1):
                                sps = pss.tile([128, 2, 512], F32, tag="s",
                                               name="sps")
                                et = attnp.tile([128, 2, 512], BF16, tag="et",
                                                name="et")
                                sets.append((g, sps, et))
                                for u in range(2):
                                    kb = 2 * g + u
                                    nc.tensor.matmul(
                                        sps[:, u, :],
                                        kT[ro:ro + HD, mc, ts(kb, 128)],
                                        qT[ro:ro + HD, mc, ts(qb, 512)],
                                        start=True, stop=True)
                                    j = kb - 4 * qb
                                    if j >= 0:
                                        nc.vector.tensor_add(sps[:, u, :],
                                                             sps[:, u, :],
                                                             mb[:, j, :])
                            for g, sps, et in sets:
                                nc.scalar.activation(out=et, in_=sps, func=EXP,
                                                     scale=0.125)
                            for g, sps, et in sets:
                                for u in range(2):
                                    kb = 2 * g + u
                                    nc.tensor.matmul(cps[0:HD + 1, :],
                                                     vaug[:, h, kb, :],
                                                     et[:, u, :],
                                                     start=(kb == 0),
                                                     stop=(kb == last_kb))
                        # flush one pending epilogue, then defer this slot's
                        while len(deferred) >= 2:
                            deferred.pop(0)()
                        deferred.append(norm_epilogue(qb, h, cps))
                        # One dense 9-matmul projection burst after each
                        # heads-0/1 slot keeps the PE HAM window busy (warm
                        # clock); their heads-2/3 consumers are a half-section
                        # away so the bursts never serialize the pipeline.
                        if h <= 1 and qk_units:
                            qk_units.pop(0)()
                        if h == 3 and qb > 0:
                            deferred.append(outproj(qb - 1))
                for fn in deferred:
                    fn()
                outproj(NQB - 1)()

    _split_multi_waits(nc)
    return nc


_NC_CACHE = []


def _get_nc():
    if not _NC_CACHE:
        _NC_CACHE.append(_build())
    return _NC_CACHE[0]


def _staircase_mask() -> np.ndarray:
    """mb[p, j, f] = 0 where k<=q for diagonal tile j, else NEG.
    Allowed iff p <= f - 128*j (q = qb*512+f, k = qb*512+128*j+p)."""
    p = np.arange(128)[:, None, None]
    j = np.arange(4)[None, :, None]
    f = np.arange(512)[None, None, :]
    return np.where(p <= f - 128 * j, 0.0, NEG).astype(np.float32)


def _in_maps(inputs: dict) -> list[dict]:
    bf16 = ml_dtypes.bfloat16
    x = np.asarray(inputs["hidden_states"], dtype=np.float32).astype(bf16)
    Wq = np.asarray(inputs["Wq"], dtype=np.float32).astype(bf16)
    Wk = np.asarray(inputs["Wk"], dtype=np.float32).astype(bf16)
    Wv = np.asarray(inputs["Wv"], dtype=np.float32).astype(bf16)
    Wo = np.asarray(inputs["Wo"], dtype=np.float32).astype(bf16)
    bq = np.asarray(inputs["bq"], dtype=np.float32).astype(bf16)
    bk = np.asarray(inputs["bk"], dtype=np.float32).astype(bf16)
    bv = np.asarray(inputs["bv"], dtype=np.float32).astype(bf16)

    xts = [np.ascontiguousarray(x[b].T) for b in range(B)]
    mb = _staircase_mask()
    maps = []
    for c in range(NCORES):
        b, hg = c // 4, c % 4
        hs = slice(hg * HSW, (hg + 1) * HSW)
        maps.append({
            "xt": xts[b],
            "wq": np.ascontiguousarray(Wq[hs, :].T),
            "wk": np.ascontiguousarray(Wk[hs, :].T),
            "wv": np.ascontiguousarray(Wv[hs, :].T),
            "wo": np.ascontiguousarray(Wo[:, hs].T),
            "bqkv": np.ascontiguousarray(np.stack([bq[hs], bk[hs], bv[hs]])),
            "mb": mb,
        })
    return maps


def run(inputs: dict, **spmd_kwargs):
    """Returns (full_output, BassKernelResults)."""
    nc = _get_nc()
    res = run_bass_kernel_spmd(nc, _in_maps(inputs), list(range(NCORES)),
                               **spmd_kwargs)
    bo = np.asarray(inputs["bo"], dtype=np.float32)
    out = np.empty((B, S, H), dtype=np.float32)
    for b in range(B):
        acc = res.results[4 * b]["out"].astype(np.float32)
        for hg in range(1, 4):
            acc = acc + res.results[4 * b + hg]["out"]
        out[b] = acc + bo
    return out, res


def kernel(**inputs) -> np.ndarray:
    out, _ = run(inputs)
    return out



# revision 11
# speedup vs baseline: 1.5959x; 1.1523x over previous
"""Causal multi-head attention (B=2, S=2048, H=1024, 16 heads, hd=64) on 8
Trainium2 NeuronCores.

Sharding: batch x head-group. Core c handles batch c//4 and the 4 heads
4*(c%4)..4*(c%4)+3 (a 256-wide column slice of Q/K/V). Each core computes its
heads' contribution to the output projection (row-parallel Wo); the host sums
the 4 partials per batch and adds bo.

Per-core kernel (all matmuls in bf16, fp32 PSUM accumulation):
  phase 1: a short K=1 warm-up matmul burst runs while the first DMAs land
           (HAM clock). q/k projections run kc-major (4 PSUM banks hold the
           4 token blocks) so each whole-row xt DMA is consumed as it
           arrives; all DMA layouts are host-prepared so per-partition runs
           are 4KB. xt stays resident in SBUF for the whole kernel.
  phase 2: per (head, 512-query block): scoresT[k,q] tiles on PE (causal:
           only k-blocks <= block end). Diagonal k-tiles are computed only
           on their valid query range (free-dim sliced), so masking reduces
           to one 128x128 triangle add on DVE per diagonal tile; exp on ACT
           (scale=1/8 folded in; scores are bounded ~|3.8| so no
           max-subtraction is needed), then ctxT[65,q] = v_aug.T @ expT
           accumulated on PE - v_aug carries a ones column so row 64 is the
           softmax denominator. Reciprocal row is broadcast across 64
           partitions with a K=1 matmul and applied on DVE, writing
           normalized ctxT straight into the outproj stationary layout.
  phase 3: out_partial[tok,1024] = ctxT.T @ WoT-slice, streamed to DRAM.
"""
import ml_dtypes
import numpy as np

import concourse.bass as bass
import concourse.mybir as mybir
import concourse.tile as tile
from concourse.bass import ts
from concourse.bass_utils import run_bass_kernel_spmd

B, S, H, NH, HD = 2, 2048, 1024, 16, 64
NCORES = 8
HPC = 4            # heads per core
HSW = HPC * HD     # 256: head-slice width
F32 = mybir.dt.float32
BF16 = mybir.dt.bfloat16
NEG = -1.0e9


def _split_multi_waits(nc) -> int:
    """This walrus accepts at most ONE sync wait per instruction. Split any
    multi-wait instruction into single-wait NOPs (same engine, just before
    it) + the instruction carrying the last wait. Equivalent semantics:
    waits run in program order on the engine's queue."""
    n = 0
    for f in nc.m.functions:
        for blk in f.blocks:
            new_insts = []
            for inst in blk.instructions:
                si = inst.sync_info
                if si is not None and si.on_wait and len(si.on_wait) > 1:
                    waits = list(si.on_wait)
                    for i, w in enumerate(waits[:-1]):
                        new_insts.append(mybir.InstNoOp(
                            name=f"{inst.name}-ws{i}",
                            engine=inst.engine,
                            bass_nofuse=True,
                            sync_info=mybir.SyncInfo(on_wait=[w], on_update=[]),
                        ))
                        n += 1
                    si.on_wait = [waits[-1]]
                new_insts.append(inst)
            blk.instructions[:] = new_insts
    return n


def _build():
    nc = bass.Bass()
    xt_d = nc.dram_tensor("xt", [H, S], BF16, kind="ExternalInput")
    wq_d = nc.dram_tensor("wq", [128, 8, HSW], BF16, kind="ExternalInput")
    wk_d = nc.dram_tensor("wk", [128, 8, HSW], BF16, kind="ExternalInput")
    wv_d = nc.dram_tensor("wv", [128, 8, HSW], BF16, kind="ExternalInput")
    wo_d = nc.dram_tensor("wo", [128, 2, H], BF16, kind="ExternalInput")
    bqkv_d = nc.dram_tensor("bqkv", [1, 3, HSW], BF16, kind="ExternalInput")
    mb_d = nc.dram_tensor("mb", [128, 128], F32, kind="ExternalInput")
    out_d = nc.dram_tensor("out", [S, H], F32, kind="ExternalOutput")

    EXP = mybir.ActivationFunctionType.Exp
    NQB = S // 512      # 4 query blocks per head
    NTC = S // 128      # 16 token chunks

    with tile.TileContext(nc) as tc:
        with tc.tile_pool(name="const", bufs=1) as constp, \
             tc.tile_pool(name="persist", bufs=1) as pers:
            wq = constp.tile([128, 8, HSW], BF16)
            wk = constp.tile([128, 8, HSW], BF16)
            wv = constp.tile([128, 8, HSW], BF16)
            wo = constp.tile([128, 2, H], BF16)
            bqkv = constp.tile([1, 3, HSW], BF16)
            mbt = constp.tile([128, 128], F32)
            onesf = constp.tile([128, 512], F32)
            nc.vector.memset(onesf, 1.0)
            ones = constp.tile([1, 512], BF16)
            nc.vector.tensor_copy(out=ones, in_=onesf[0:1, :])

            xt = pers.tile([128, 8, S], BF16)     # resident for whole kernel
            qT = pers.tile([128, 2, S], BF16)     # [2 heads x 64 hd, mchunk, tok]
            kT = pers.tile([128, 2, S], BF16)
            vaug = pers.tile([128, 4, NTC, HD + 1], BF16)  # [ktok, head, kchunk, hd|1]
            ctxT = pers.tile([128, 2, S], BF16)   # outproj stationary layout
            nc.vector.tensor_copy(
                out=vaug[:, :, :, HD:HD + 1],
                in_=onesf[:, 0:64].rearrange("p (a b o) -> p a b o", a=4, b=16))

            # ---- phase 1: projections ----
            with tc.tile_pool(name="ps1", bufs=4, space="PSUM") as ps1, \
                 tc.tile_pool(name="ps1v", bufs=3, space="PSUM") as ps1v, \
                 tc.tile_pool(name="psw", bufs=1, space="PSUM") as psw:
                # xt whole rows per kc: contiguous 4KB per-partition runs.
                # kc=0 + wq first so the q-pass can start ASAP.
                nc.sync.dma_start(out=xt[:, 0, :], in_=xt_d[ts(0, 128), :])
                nc.sync.dma_start(out=wq, in_=wq_d[:, :, :])
                for kc in range(1, 8):
                    nc.sync.dma_start(out=xt[:, kc, :], in_=xt_d[ts(kc, 128), :])
                nc.sync.dma_start(out=wk, in_=wk_d[:, :, :])
                nc.sync.dma_start(out=bqkv, in_=bqkv_d[:, :, :])
                nc.sync.dma_start(out=wv, in_=wv_d[:, :, :])
                nc.sync.dma_start(out=mbt, in_=mb_d[:, :])
                nc.sync.dma_start(out=wo, in_=wo_d[:, :, :])

                # PE warm-up: cheap K=1/M=1 matmuls tick the HAM activity
                # window while the first DMAs land.
                wps = psw.tile([1, 512], F32, tag="w", name="wps")
                for _ in range(12):
                    nc.tensor.matmul(wps, ones[0:1, 0:1], ones[0:1, :],
                                     start=True, stop=True)

                # q/k projections kc-major: each whole-row xt DMA is consumed
                # as it arrives; 4 PSUM banks hold the 4 token blocks.
                for w, brow, dst in ((wq, 0, qT), (wk, 1, kT)):
                    pb = [ps1.tile([128, 512], F32, tag="ps1", name="pb")
                          for _ in range(NQB)]
                    for kc in range(8):
                        for nb in range(NQB):
                            nc.tensor.matmul(pb[nb], w[:, kc, 0:128],
                                             xt[:, kc, ts(nb, 512)],
                                             start=(kc == 0), stop=False)
                    for nb in range(NQB):
                        nc.tensor.matmul(pb[nb], bqkv[0:1, brow, 0:128],
                                         ones[0:1, 0:512], start=False, stop=True)
                        nc.vector.tensor_copy(out=dst[:, 0, ts(nb, 512)],
                                              in_=pb[nb])

                for t in range(NTC):
                    ps = ps1v.tile([128, HSW], F32, tag="psv", name="ps")
                    for kc in range(8):
                        nc.tensor.matmul(ps, xt[:, kc, ts(t, 128)], wv[:, kc, :],
                                         start=(kc == 0), stop=False)
                    nc.tensor.matmul(ps, ones[0:1, 0:128], bqkv[0:1, 2, :],
                                     start=False, stop=True)
                    for h in range(HPC):
                        nc.vector.tensor_copy(out=vaug[:, h, t, 0:HD],
                                              in_=ps[:, ts(h, HD)])

            # ---- phase 2+3: attention with software-pipelined epilogues ----
            # Heads processed singly (qb outer). The normalization epilogue of
            # slot (qb, h) -- reciprocal via ACT exp(-ln d), PE broadcast, DVE
            # scale -- is DEFERRED until after the NEXT slot's score/ctx groups
            # are emitted: the static per-engine schedule then has the next
            # slot's matmuls between ctx-stop and the broadcast matmul, so the
            # PE never idles waiting on the reciprocal chain. Outproj for qb is
            # deferred two slots for the same reason.
            # Diagonal k-tiles are free-dim sliced to their valid query range
            # [128j, 512): only a 128x128 triangle add remains for masking.
            # PSUM: pss 2x2 + ctx 2 + misc 1 + ps1b 1 = 8 banks.
            with tc.tile_pool(name="pss", bufs=2, space="PSUM") as pss, \
                 tc.tile_pool(name="psc", bufs=2, space="PSUM") as psc, \
                 tc.tile_pool(name="psm", bufs=1, space="PSUM") as psm, \
                 tc.tile_pool(name="ps1b", bufs=1, space="PSUM") as ps1b, \
                 tc.tile_pool(name="attnp", bufs=3) as attnp, \
                 tc.tile_pool(name="outp", bufs=3) as outp:

                # Interleaved heads-2/3 q/k projection chunks: dense 9-matmul
                # accumulation runs that keep the PE HAM activity window busy
                # (warm clock) between attention slots. xt is resident, so no
                # re-DMA is needed.
                def qk_mc1_chunk(w, brow, dst, nb):
                    def run():
                        ps = ps1b.tile([128, 512], F32, tag="ps1b", name="ps1b")
                        for kc in range(8):
                            nc.tensor.matmul(ps, w[:, kc, 128:256],
                                             xt[:, kc, ts(nb, 512)],
                                             start=(kc == 0), stop=False)
                        nc.tensor.matmul(ps, bqkv[0:1, brow, 128:256],
                                         ones[0:1, 0:512], start=False, stop=True)
                        nc.vector.tensor_copy(out=dst[:, 1, ts(nb, 512)], in_=ps)
                    return run
                qk_units = [qk_mc1_chunk(w, brow, dst, nb)
                            for (w, brow, dst) in ((wq, 0, qT), (wk, 1, kT))
                            for nb in range(NQB)]

                def norm_epilogue(qb, h, cps):
                    def run():
                        mc, ro = h // 2, (h % 2) * HD
                        lnr = attnp.tile([1, 512], F32, tag="lnr", name="lnr")
                        nc.scalar.activation(out=lnr, in_=cps[HD:HD + 1, :],
                                             func=mybir.ActivationFunctionType.Ln)
                        rec = attnp.tile([1, 512], BF16, tag="rec", name="rec")
                        nc.scalar.activation(out=rec, in_=lnr, func=EXP,
                                             scale=-1.0)
                        bps = psm.tile([128, 512], F32, tag="m", name="bps")
                        nc.tensor.matmul(bps[0:HD, :], ones[0:1, 0:HD],
                                         rec[0:1, :], start=True, stop=True)
                        bsb = attnp.tile([HD, 512], BF16, tag="bsb", name="bsb")
                        nc.vector.tensor_copy(out=bsb, in_=bps[0:HD, :])
                        nc.vector.tensor_mul(
                            out=ctxT[ro:ro + HD, mc, ts(qb, 512)],
                            in0=cps[0:HD, :], in1=bsb)
                    return run

                def outproj(qb):
                    def run():
                        for t in range(4 * qb, 4 * qb + 4):
                            osb = outp.tile([128, H], F32, tag="osb", name="osb")
                            for n2 in range(2):
                                # Alternate across two PSUM banks (ps1b is idle
                                # once the qk bursts are done) so each outproj
                                # blob is a dense 8-matmul run - it both flows
                                # faster and re-warms the HAM clock.
                                ops = (psm if n2 == 0 else ps1b).tile(
                                    [128, 512], F32,
                                    tag="m" if n2 == 0 else "ps1b", name="ops")
                                nc.tensor.matmul(ops, ctxT[:, 0, ts(t, 128)],
                                                 wo[:, 0, ts(n2, 512)],
                                                 start=True, stop=False)
                                nc.tensor.matmul(ops, ctxT[:, 1, ts(t, 128)],
                                                 wo[:, 1, ts(n2, 512)],
                                                 start=False, stop=True)
                                nc.vector.tensor_copy(out=osb[:, ts(n2, 512)],
                                                      in_=ops)
                            nc.sync.dma_start(out=out_d[ts(t, 128), :], in_=osb)
                    return run

                deferred = []
                for qb, h in ([(q, hh) for q in range(NQB) for hh in (0, 1)]
                              + [(q, hh) for q in range(NQB) for hh in (2, 3)]):
                    last_kb = 4 * qb + 3
                    if True:
                        mc, ro = h // 2, (h % 2) * HD
                        cps = psc.tile([128, 512], F32, tag="ctx", name="cps")
                        # Emit group PAIRS: 4 scores mms, 2 exps, 4 ctx mms.
                        # The PE stream becomes continuous ~8-matmul dense runs
                        # (scores of pair N overlap exp of pair N-1), which
                        # keeps the HAM activity window busy (full clock).
                        for gp in range(qb + 1):
                            diag = (gp == qb)
                            sets = []
                            for g in (2 * gp, 2 * gp + 1):
                                sps = pss.tile([128, 2, 512], F32, tag="s",
                                               name="sps")
                                et = attnp.tile([128, 2, 512], BF16, tag="et",
                                                name="et")
                                sets.append((g, sps, et))
                                for u in range(2):
                                    kb = 2 * g + u
                                    j = kb - 4 * qb
                                    lo = 128 * j if j > 0 else 0
                                    nc.tensor.matmul(
                                        sps[:, u, lo:512],
                                        kT[ro:ro + HD, mc, ts(kb, 128)],
                                        qT[ro:ro + HD, mc,
                                           qb * 512 + lo:(qb + 1) * 512],
                                        start=True, stop=True)
                                    if j >= 0:
                                        nc.vector.tensor_add(
                                            sps[:, u, 128 * j:128 * j + 128],
                                            sps[:, u, 128 * j:128 * j + 128],
                                            mbt)
                            for g, sps, et in sets:
                                if diag:
                                    for u in range(2):
                                        kb = 2 * g + u
                                        j = kb - 4 * qb
                                        lo = 128 * j if j > 0 else 0
                                        nc.scalar.activation(
                                            out=et[:, u, lo:512],
                                            in_=sps[:, u, lo:512],
                                            func=EXP, scale=0.125)
                                else:
                                    nc.scalar.activation(out=et, in_=sps,
                                                         func=EXP, scale=0.125)
                            for g, sps, et in sets:
                                for u in range(2):
                                    kb = 2 * g + u
                                    j = kb - 4 * qb
                                    lo = 128 * j if j > 0 else 0
                                    nc.tensor.matmul(
                                        cps[0:HD + 1, lo:512],
                                        vaug[:, h, kb, :],
                                        et[:, u, lo:512],
                                        start=(kb == 0),
                                        stop=(kb == last_kb),
                                        skip_group_check=True)
                        # flush one pending epilogue, then defer this slot's
                        while len(deferred) >= 2:
                            deferred.pop(0)()
                        deferred.append(norm_epilogue(qb, h, cps))
                        # One dense 9-matmul projection burst after each
                        # heads-0/1 slot keeps the PE HAM window busy (warm
                        # clock); their heads-2/3 consumers are a half-section
                        # away so the bursts never serialize the pipeline.
                        if h <= 1 and qk_units:
                            qk_units.pop(0)()
                        if h == 3 and qb > 0:
                            deferred.append(outproj(qb - 1))
                for fn in deferred:
                    fn()
                outproj(NQB - 1)()

    _split_multi_waits(nc)
    return nc


_NC_CACHE = []


def _get_nc():
    if not _NC_CACHE:
        _NC_CACHE.append(_build())
    return _NC_CACHE[0]


def _triangle_mask() -> np.ndarray:
    """mbt[p, f] = 0 where p <= f (key p attends to query f), else NEG.
    Applied to the 128x128 leading-diagonal corner of each diagonal k-tile."""
    p = np.arange(128)[:, None]
    f = np.arange(128)[None, :]
    return np.where(p <= f, 0.0, NEG).astype(np.float32)


def _in_maps(inputs: dict) -> list[dict]:
    bf16 = ml_dtypes.bfloat16
    x = np.asarray(inputs["hidden_states"], dtype=np.float32).astype(bf16)
    Wq = np.asarray(inputs["Wq"], dtype=np.float32).astype(bf16)
    Wk = np.asarray(inputs["Wk"], dtype=np.float32).astype(bf16)
    Wv = np.asarray(inputs["Wv"], dtype=np.float32).astype(bf16)
    Wo = np.asarray(inputs["Wo"], dtype=np.float32).astype(bf16)
    bq = np.asarray(inputs["bq"], dtype=np.float32).astype(bf16)
    bk = np.asarray(inputs["bk"], dtype=np.float32).astype(bf16)
    bv = np.asarray(inputs["bv"], dtype=np.float32).astype(bf16)

    xts = [np.ascontiguousarray(x[b].T) for b in range(B)]
    mbt = _triangle_mask()

    def wlayout(wt, c):
        # [c*128, n] -> [128, c, n] so per-partition DMA runs are contiguous
        return np.ascontiguousarray(
            wt.reshape(c, 128, wt.shape[1]).transpose(1, 0, 2))

    maps = []
    for c in range(NCORES):
        b, hg = c // 4, c % 4
        hs = slice(hg * HSW, (hg + 1) * HSW)
        maps.append({
            "xt": xts[b],
            "wq": wlayout(np.ascontiguousarray(Wq[hs, :].T), 8),
            "wk": wlayout(np.ascontiguousarray(Wk[hs, :].T), 8),
            "wv": wlayout(np.ascontiguousarray(Wv[hs, :].T), 8),
            "wo": wlayout(np.ascontiguousarray(Wo[:, hs].T), 2),
            "bqkv": np.ascontiguousarray(
                np.stack([bq[hs], bk[hs], bv[hs]])[None]),
            "mb": mbt,
        })
    return maps


def run(inputs: dict, **spmd_kwargs):
    """Returns (full_output, BassKernelResults)."""
    nc = _get_nc()
    res = run_bass_kernel_spmd(nc, _in_maps(inputs), list(range(NCORES)),
                               **spmd_kwargs)
    bo = np.asarray(inputs["bo"], dtype=np.float32)
    out = np.empty((B, S, H), dtype=np.float32)
    for b in range(B):
        acc = res.results[4 * b]["out"].astype(np.float32)
        for hg in range(1, 4):
            acc = acc + res.results[4 * b + hg]["out"]
        out[b] = acc + bo
    return out, res


def kernel(**inputs) -> np.ndarray:
    out, _ = run(inputs)
    return out


# revision 21
# speedup vs baseline: 1.6490x; 1.0333x over previous
"""Causal multi-head attention (B=2, S=2048, H=1024, 16 heads, hd=64) on 8
Trainium2 NeuronCores.

Sharding: batch x head-group. Core c handles batch c//4 and the 4 heads
4*(c%4)..4*(c%4)+3 (a 256-wide column slice of Q/K/V). Each core computes its
heads' contribution to the output projection (row-parallel Wo); the host sums
the 4 partials per batch and adds bo.

Per-core kernel (all matmuls in bf16, fp32 PSUM accumulation):
  phase 1: a short K=1 warm-up matmul burst runs while the first DMAs land
           (HAM clock). q/k projections run kc-major (4 PSUM banks hold the
           4 token blocks) so each whole-row xt DMA is consumed as it
           arrives; all DMA layouts are host-prepared so per-partition runs
           are 4KB. xt stays resident in SBUF for the whole kernel.
  phase 2: per (head, 512-query block): scoresT[k,q] tiles on PE (causal:
           only k-blocks <= block end). Diagonal k-tiles are computed only
           on their valid query range (free-dim sliced), so masking reduces
           to one 128x128 triangle add on DVE per diagonal tile; exp on ACT
           (scale=1/8 folded in; scores are bounded ~|3.8| so no
           max-subtraction is needed), then ctxT[65,q] = v_aug.T @ expT
           accumulated on PE - v_aug carries a ones column so row 64 is the
           softmax denominator. Reciprocal row is broadcast across 64
           partitions with a K=1 matmul and applied on DVE, writing
           normalized ctxT straight into the outproj stationary layout.
  phase 3: out_partial[tok,1024] = ctxT.T @ WoT-slice, streamed to DRAM.
"""
import ml_dtypes
import numpy as np

import concourse.bass as bass
import concourse.mybir as mybir
import concourse.tile as tile
from concourse.bass import ts
from concourse.bass_utils import run_bass_kernel_spmd

B, S, H, NH, HD = 2, 2048, 1024, 16, 64
NCORES = 8
HPC = 4            # heads per core
HSW = HPC * HD     # 256: head-slice width
F32 = mybir.dt.float32
BF16 = mybir.dt.bfloat16
NEG = -1.0e9


def _split_multi_waits(nc) -> int:
    """This walrus accepts at most ONE sync wait per instruction. Split any
    multi-wait instruction into single-wait NOPs (same engine, just before
    it) + the instruction carrying the last wait. Equivalent semantics:
    waits run in program order on the engine's queue."""
    n = 0
    for f in nc.m.functions:
        for blk in f.blocks:
            new_insts = []
            for inst in blk.instructions:
                si = inst.sync_info
                if si is not None and si.on_wait and len(si.on_wait) > 1:
                    waits = list(si.on_wait)
                    for i, w in enumerate(waits[:-1]):
                        new_insts.append(mybir.InstNoOp(
                            name=f"{inst.name}-ws{i}",
                            engine=inst.engine,
                            bass_nofuse=True,
                            sync_info=mybir.SyncInfo(on_wait=[w], on_update=[]),
                        ))
                        n += 1
                    si.on_wait = [waits[-1]]
                new_insts.append(inst)
            blk.instructions[:] = new_insts
    return n


def _build():
    nc = bass.Bass()
    xt_d = nc.dram_tensor("xt", [H, S], BF16, kind="ExternalInput")
    wq_d = nc.dram_tensor("wq", [128, 8, HSW], BF16, kind="ExternalInput")
    wk_d = nc.dram_tensor("wk", [128, 8, HSW], BF16, kind="ExternalInput")
    wv_d = nc.dram_tensor("wv", [128, 8, HSW], BF16, kind="ExternalInput")
    wo_d = nc.dram_tensor("wo", [128, 2, H], BF16, kind="ExternalInput")
    bqkv_d = nc.dram_tensor("bqkv", [1, 3, HSW], BF16, kind="ExternalInput")
    bqkvt_d = nc.dram_tensor("bqkvt", [128, 2, 2], F32, kind="ExternalInput")
    mb_d = nc.dram_tensor("mb", [128, 128], F32, kind="ExternalInput")
    out_d = nc.dram_tensor("out", [S, H], F32, kind="ExternalOutput")

    EXP = mybir.ActivationFunctionType.Exp
    NQB = S // 512      # 4 query blocks per head
    NTC = S // 128      # 16 token chunks

    with tile.TileContext(nc) as tc:
        with tc.tile_pool(name="const", bufs=1) as constp, \
             tc.tile_pool(name="persist", bufs=1) as pers:
            wq = constp.tile([128, 8, HSW], BF16)
            wk = constp.tile([128, 8, HSW], BF16)
            wv = constp.tile([128, 8, HSW], BF16)
            wo = constp.tile([128, 2, H], BF16)
            bqkv = constp.tile([1, 3, HSW], BF16)
            bqkvt = constp.tile([128, 2, 2], F32)  # [p, mc, q|k] per-row bias
            mbt = constp.tile([128, 128], F32)
            onesf = constp.tile([128, 512], F32)
            nc.vector.memset(onesf, 1.0)
            ones = constp.tile([1, 512], BF16)
            nc.vector.tensor_copy(out=ones, in_=onesf[0:1, :])

            xt = pers.tile([128, 8, S], BF16)     # resident for whole kernel
            qT = pers.tile([128, 2, S], BF16)     # [2 heads x 64 hd, mchunk, tok]
            kT = pers.tile([128, 2, S], BF16)
            vaug = pers.tile([128, 4, NTC, HD + 1], BF16)  # [ktok, head, kchunk, hd|1]
            ctxT = pers.tile([128, 2, S], BF16)   # outproj stationary layout
            nc.vector.tensor_copy(
                out=vaug[:, :, :, HD:HD + 1],
                in_=onesf[:, 0:64].rearrange("p (a b o) -> p a b o", a=4, b=16))

            # ---- phase 1: projections ----
            with tc.tile_pool(name="ps1", bufs=4, space="PSUM") as ps1, \
                 tc.tile_pool(name="ps1v", bufs=3, space="PSUM") as ps1v, \
                 tc.tile_pool(name="psw", bufs=1, space="PSUM") as psw:
                # xt whole rows per kc: contiguous 4KB per-partition runs.
                # kc=0 + wq first so the q-pass can start ASAP.
                nc.sync.dma_start(out=xt[:, 0, :], in_=xt_d[ts(0, 128), :])
                nc.sync.dma_start(out=wq, in_=wq_d[:, :, :])
                for kc in range(1, 8):
                    nc.sync.dma_start(out=xt[:, kc, :], in_=xt_d[ts(kc, 128), :])
                nc.sync.dma_start(out=wk, in_=wk_d[:, :, :])
                nc.sync.dma_start(out=bqkv, in_=bqkv_d[:, :, :])
                nc.sync.dma_start(out=bqkvt, in_=bqkvt_d[:, :, :])
                nc.sync.dma_start(out=wv, in_=wv_d[:, :, :])
                nc.sync.dma_start(out=mbt, in_=mb_d[:, :])
                nc.sync.dma_start(out=wo, in_=wo_d[:, :, :])

                # PE warm-up: cheap K=1/M=1 matmuls tick the HAM activity
                # window while the first DMAs land.
                wps = psw.tile([1, 512], F32, tag="w", name="wps")
                for _ in range(12):
                    nc.tensor.matmul(wps, ones[0:1, 0:1], ones[0:1, :],
                                     start=True, stop=True)

                # q/k projections kc-major: each whole-row xt DMA is consumed
                # as it arrives; 4 PSUM banks hold the 4 token blocks.
                for w, brow, dst in ((wq, 0, qT), (wk, 1, kT)):
                    pb = [ps1.tile([128, 512], F32, tag="ps1", name="pb")
                          for _ in range(NQB)]
                    for kc in range(8):
                        for nb in range(NQB):
                            nc.tensor.matmul(pb[nb], w[:, kc, 0:128],
                                             xt[:, kc, ts(nb, 512)],
                                             start=(kc == 0), stop=(kc == 7))
                    for nb in range(NQB):
                        # bias folded into the PSUM->SBUF cast (per-partition
                        # scalar add on DVE)
                        nc.vector.tensor_scalar_add(
                            out=dst[:, 0, ts(nb, 512)], in0=pb[nb],
                            scalar1=bqkvt[:, 0, brow:brow + 1])

                for t in range(NTC):
                    ps = ps1v.tile([128, HSW], F32, tag="psv", name="ps")
                    for kc in range(8):
                        nc.tensor.matmul(ps, xt[:, kc, ts(t, 128)], wv[:, kc, :],
                                         start=(kc == 0), stop=False)
                    nc.tensor.matmul(ps, ones[0:1, 0:128], bqkv[0:1, 2, :],
                                     start=False, stop=True)
                    for h in range(HPC):
                        nc.vector.tensor_copy(out=vaug[:, h, t, 0:HD],
                                              in_=ps[:, ts(h, HD)])

            # ---- phase 2+3: attention with software-pipelined epilogues ----
            # Heads processed singly (qb outer). The normalization epilogue of
            # slot (qb, h) -- reciprocal via ACT exp(-ln d), PE broadcast, DVE
            # scale -- is DEFERRED until after the NEXT slot's score/ctx groups
            # are emitted: the static per-engine schedule then has the next
            # slot's matmuls between ctx-stop and the broadcast matmul, so the
            # PE never idles waiting on the reciprocal chain. Outproj for qb is
            # deferred two slots for the same reason.
            # Diagonal k-tiles are free-dim sliced to their valid query range
            # [128j, 512): only a 128x128 triangle add remains for masking.
            # PSUM: pss 2x2 + ctx 2 + misc 1 + ps1b 1 = 8 banks.
            with tc.tile_pool(name="pss", bufs=2, space="PSUM") as pss, \
                 tc.tile_pool(name="psc", bufs=2, space="PSUM") as psc, \
                 tc.tile_pool(name="psm", bufs=1, space="PSUM") as psm, \
                 tc.tile_pool(name="ps1b", bufs=1, space="PSUM") as ps1b, \
                 tc.tile_pool(name="attnp", bufs=3) as attnp, \
                 tc.tile_pool(name="outp", bufs=3) as outp:

                # Interleaved heads-2/3 q/k projection chunks: dense 9-matmul
                # accumulation runs that keep the PE HAM activity window busy
                # (warm clock) between attention slots. xt is resident, so no
                # re-DMA is needed.
                def qk_mc1_chunk(w, brow, dst, nb):
                    def run():
                        ps = ps1b.tile([128, 512], F32, tag="ps1b", name="ps1b")
                        for kc in range(8):
                            nc.tensor.matmul(ps, w[:, kc, 128:256],
                                             xt[:, kc, ts(nb, 512)],
                                             start=(kc == 0), stop=(kc == 7))
                        nc.vector.tensor_scalar_add(
                            out=dst[:, 1, ts(nb, 512)], in0=ps,
                            scalar1=bqkvt[:, 1, brow:brow + 1])
                    return run
                qk_units = [qk_mc1_chunk(w, brow, dst, nb)
                            for (w, brow, dst) in ((wq, 0, qT), (wk, 1, kT))
                            for nb in range(NQB)]

                def norm_epilogue(qb, h, cps):
                    def run():
                        mc, ro = h // 2, (h % 2) * HD
                        lnr = attnp.tile([1, 512], F32, tag="lnr", name="lnr")
                        nc.scalar.activation(out=lnr, in_=cps[HD:HD + 1, :],
                                             func=mybir.ActivationFunctionType.Ln)
                        rec = attnp.tile([1, 512], BF16, tag="rec", name="rec")
                        nc.scalar.activation(out=rec, in_=lnr, func=EXP,
                                             scale=-1.0)
                        bps = psm.tile([128, 512], F32, tag="m", name="bps")
                        nc.tensor.matmul(bps[0:HD, :], ones[0:1, 0:HD],
                                         rec[0:1, :], start=True, stop=True)
                        bsb = attnp.tile([HD, 512], BF16, tag="bsb", name="bsb")
                        nc.vector.tensor_copy(out=bsb, in_=bps[0:HD, :])
                        nc.vector.tensor_mul(
                            out=ctxT[ro:ro + HD, mc, ts(qb, 512)],
                            in0=cps[0:HD, :], in1=bsb)
                    return run

                def outproj(qb):
                    def run():
                        for t in range(4 * qb, 4 * qb + 4):
                            osb = outp.tile([128, H], F32, tag="osb", name="osb")
                            for n2 in range(2):
                                # Alternate across two PSUM banks (ps1b is idle
                                # once the qk bursts are done) so each outproj
                                # blob is a dense 8-matmul run - it both flows
                                # faster and re-warms the HAM clock.
                                ops = (psm if n2 == 0 else ps1b).tile(
                                    [128, 512], F32,
                                    tag="m" if n2 == 0 else "ps1b", name="ops")
                                nc.tensor.matmul(ops, ctxT[:, 0, ts(t, 128)],
                                                 wo[:, 0, ts(n2, 512)],
                                                 start=True, stop=False)
                                nc.tensor.matmul(ops, ctxT[:, 1, ts(t, 128)],
                                                 wo[:, 1, ts(n2, 512)],
                                                 start=False, stop=True)
                                nc.vector.tensor_copy(out=osb[:, ts(n2, 512)],
                                                      in_=ops)
                            nc.sync.dma_start(out=out_d[ts(t, 128), :], in_=osb)
                    return run

                deferred = []
                for qb, h in ([(q, hh) for q in range(NQB) for hh in (0, 1)]
                              + [(q, hh) for q in range(NQB) for hh in (2, 3)]):
                    last_kb = 4 * qb + 3
                    if True:
                        mc, ro = h // 2, (h % 2) * HD
                        cps = psc.tile([128, 512], F32, tag="ctx", name="cps")
                        # Emit group PAIRS: 4 scores mms, 2 exps, 4 ctx mms.
                        # The PE stream becomes continuous ~8-matmul dense runs
                        # (scores of pair N overlap exp of pair N-1), which
                        # keeps the HAM activity window busy (full clock).
                        for gp in range(qb + 1):
                            diag = (gp == qb)
                            sets = []
                            for g in (2 * gp, 2 * gp + 1):
                                sps = pss.tile([128, 2, 512], F32, tag="s",
                                               name="sps")
                                et = attnp.tile([128, 2, 512], BF16, tag="et",
                                                name="et")
                                sets.append((g, sps, et))
                                for u in range(2):
                                    kb = 2 * g + u
                                    j = kb - 4 * qb
                                    lo = 128 * j if j > 0 else 0
                                    nc.tensor.matmul(
                                        sps[:, u, lo:512],
                                        kT[ro:ro + HD, mc, ts(kb, 128)],
                                        qT[ro:ro + HD, mc,
                                           qb * 512 + lo:(qb + 1) * 512],
                                        start=True, stop=True)
                                    if j >= 0:
                                        nc.vector.tensor_add(
                                            sps[:, u, 128 * j:128 * j + 128],
                                            sps[:, u, 128 * j:128 * j + 128],
                                            mbt)
                            for g, sps, et in sets:
                                if diag:
                                    for u in range(2):
                                        kb = 2 * g + u
                                        j = kb - 4 * qb
                                        lo = 128 * j if j > 0 else 0
                                        nc.scalar.activation(
                                            out=et[:, u, lo:512],
                                            in_=sps[:, u, lo:512],
                                            func=EXP, scale=0.125)
                                else:
                                    nc.scalar.activation(out=et, in_=sps,
                                                         func=EXP, scale=0.125)
                            for g, sps, et in sets:
                                for u in range(2):
                                    kb = 2 * g + u
                                    j = kb - 4 * qb
                                    lo = 128 * j if j > 0 else 0
                                    nc.tensor.matmul(
                                        cps[0:HD + 1, lo:512],
                                        vaug[:, h, kb, :],
                                        et[:, u, lo:512],
                                        start=(kb == 0),
                                        stop=(kb == last_kb),
                                        skip_group_check=True)
                        # flush one pending epilogue, then defer this slot's
                        while len(deferred) >= 2:
                            deferred.pop(0)()
                        # outproj(qb-1) queued BEFORE this slot's norm so the
                        # final flush interleaves PE-heavy outproj work with
                        # the last norm chains.
                        if h == 3 and qb > 0:
                            deferred.append(outproj(qb - 1))
                        deferred.append(norm_epilogue(qb, h, cps))
                        # One dense 9-matmul projection burst after each
                        # heads-0/1 slot keeps the PE HAM window busy (warm
                        # clock); their heads-2/3 consumers are a half-section
                        # away so the bursts never serialize the pipeline.
                        if h <= 1 and qk_units:
                            qk_units.pop(0)()
                for fn in deferred:
                    fn()
                outproj(NQB - 1)()

    _split_multi_waits(nc)
    return nc


_NC_CACHE = []


def _get_nc():
    if not _NC_CACHE:
        _NC_CACHE.append(_build())
    return _NC_CACHE[0]


def _triangle_mask() -> np.ndarray:
    """mbt[p, f] = 0 where p <= f (key p attends to query f), else NEG.
    Applied to the 128x128 leading-diagonal corner of each diagonal k-tile."""
    p = np.arange(128)[:, None]
    f = np.arange(128)[None, :]
    return np.where(p <= f, 0.0, NEG).astype(np.float32)


def _in_maps(inputs: dict) -> list[dict]:
    bf16 = ml_dtypes.bfloat16
    x = np.asarray(inputs["hidden_states"], dtype=np.float32).astype(bf16)
    Wq = np.asarray(inputs["Wq"], dtype=np.float32).astype(bf16)
    Wk = np.asarray(inputs["Wk"], dtype=np.float32).astype(bf16)
    Wv = np.asarray(inputs["Wv"], dtype=np.float32).astype(bf16)
    Wo = np.asarray(inputs["Wo"], dtype=np.float32).astype(bf16)
    bq = np.asarray(inputs["bq"], dtype=np.float32).astype(bf16)
    bk = np.asarray(inputs["bk"], dtype=np.float32).astype(bf16)
    bv = np.asarray(inputs["bv"], dtype=np.float32).astype(bf16)

    xts = [np.ascontiguousarray(x[b].T) for b in range(B)]
    mbt = _triangle_mask()

    def wlayout(wt, c):
        # [c*128, n] -> [128, c, n] so per-partition DMA runs are contiguous
        return np.ascontiguousarray(
            wt.reshape(c, 128, wt.shape[1]).transpose(1, 0, 2))

    bqf = np.asarray(inputs["bq"], dtype=np.float32)
    bkf = np.asarray(inputs["bk"], dtype=np.float32)
    maps = []
    for c in range(NCORES):
        b, hg = c // 4, c % 4
        hs = slice(hg * HSW, (hg + 1) * HSW)
        # [p, mc, q|k] fp32 per-row bias for the DVE tensor_scalar add
        bqkvt = np.ascontiguousarray(
            np.stack([bqf[hs].reshape(2, 128), bkf[hs].reshape(2, 128)],
                     axis=-1).transpose(1, 0, 2))
        maps.append({
            "xt": xts[b],
            "wq": wlayout(np.ascontiguousarray(Wq[hs, :].T), 8),
            "wk": wlayout(np.ascontiguousarray(Wk[hs, :].T), 8),
            "wv": wlayout(np.ascontiguousarray(Wv[hs, :].T), 8),
            "wo": wlayout(np.ascontiguousarray(Wo[:, hs].T), 2),
            "bqkv": np.ascontiguousarray(
                np.stack([bq[hs], bk[hs], bv[hs]])[None]),
            "bqkvt": bqkvt,
            "mb": mbt,
        })
    return maps


def run(inputs: dict, **spmd_kwargs):
    """Returns (full_output, BassKernelResults)."""
    nc = _get_nc()
    res = run_bass_kernel_spmd(nc, _in_maps(inputs), list(range(NCORES)),
                               **spmd_kwargs)
    bo = np.asarray(inputs["bo"], dtype=np.float32)
    out = np.empty((B, S, H), dtype=np.float32)
    for b in range(B):
        acc = res.results[4 * b]["out"].astype(np.float32)
        for hg in range(1, 4):
            acc = acc + res.results[4 * b + hg]["out"]
        out[b] = acc + bo
    return out, res


def kernel(**inputs) -> np.ndarray:
    out, _ = run(inputs)
    return out
